# revision 19
# baseline (speedup 1.0000x reference)
"""Trainium2 Bass kernel for full-dim attention — bf16 resident v3.

Folded algorithm (scores = x M x.T / 32 with M = wq.T wk;
out = (p x) W2.T with W2 = wo wv). All matmul operands bf16 (same PE
rate as fp32r, half the SBUF/DMA), which lets every input stay fully
resident: x.T [128,8,2048], x natural [128,16,1024], mT, w2T, uT, pT
(both halves), px. Total ~21 MB SBUF, 12 MB DMA in, all front-loaded —
no mid-kernel streaming bursts (the v2 restructure showed the fp32r
xTk/xn re-streams run at ~270 GB/s demand and stall the PE).

Stage order (PE): uT(q0) S0 uT(q1) Z0 px0 S1 out0 Z1 px1 out1 — every
cross-engine handoff (psum evac, exp, S1 sums, Z) hides behind a PE
stage that doesn't depend on it.
"""

import sys

if "/opt/trn_rl_repo" not in sys.path:
    sys.path.insert(0, "/opt/trn_rl_repo")

import numpy as np
import ml_dtypes

N_CORES = 8
P = 128

_BUILD_CACHE = {}


def _build(S, D, SQ):
    import concourse.mybir as mybir
    import concourse.tile as tile
    from concourse import bacc

    key = (S, D, SQ)
    if key in _BUILD_CACHE:
        return _BUILD_CACHE[key]

    dt = mybir.dt
    DS = D // P           # d subtiles (8)
    SKT = S // P          # key tiles (16)
    SQT = SQ // P         # query tiles (8)
    NB = 512
    NBg = min(NB, D)      # 512
    NH = max(1, SQ // NB)          # query halves (2)
    HW_ = SQ // NH                 # queries per half (512)
    HT = HW_ // P                  # query tiles per half (4)
    QB = S // NB                   # 512-query blocks over all keys (4)
    INV_SQRT_D = 1.0 / float(np.sqrt(np.float32(D)))

    nc = bacc.Bacc(None, target_bir_lowering=False, debug=False)

    bf = dt.bfloat16
    # xT is query-block-major [P, QB, DS, 512]: a chunk [:, qb, ds0:ds1, :]
    # is contiguous >=2KB per partition. 1KB-run DMA transfers move at only
    # ~12.3 GB/s/engine (81ns/packet overhead-bound); 2KB+ runs hit ~24.5,
    # i.e. ~390 GB/s aggregate over the 16 engines.
    xT_d = nc.dram_tensor("xT", [P, QB, DS, NB], bf, kind="ExternalInput")
    xn_d = nc.dram_tensor("xn", [P, SKT, D], bf, kind="ExternalInput")
    # mT is col-half-major [P, 2, DS, 512]: uT group g consumes col-half g
    # across all ds, so chunk [:, h, ds0:ds1, :] is both contiguous (>=2KB
    # runs) and exactly what the next group needs.
    mT_d = nc.dram_tensor("mT", [P, 2, DS, NB], bf, kind="ExternalInput")
    w2T_d = nc.dram_tensor("w2T", [P, DS, D], bf, kind="ExternalInput")
    y_d = nc.dram_tensor("y", [SQ, D], dt.float32, kind="ExternalOutput")

    with tile.TileContext(nc) as tc:
        with (
            tc.tile_pool(name="xT", bufs=1) as xT_pool,
            tc.tile_pool(name="xn", bufs=1) as xn_pool,
            tc.tile_pool(name="mT", bufs=1) as mT_pool,
            tc.tile_pool(name="w2T", bufs=1) as w2T_pool,
            tc.tile_pool(name="uT", bufs=1) as uT_pool,
            tc.tile_pool(name="pT", bufs=1) as pT_pool,
            tc.tile_pool(name="px", bufs=1) as px_pool,
            tc.tile_pool(name="stat", bufs=1) as stat_pool,
            tc.tile_pool(name="outsb", bufs=4) as out_pool,
            tc.tile_pool(name="ps", bufs=6, space="PSUM") as ps_pool,
            tc.tile_pool(name="zps", bufs=2, space="PSUM") as z_pool,
        ):
            xT = xT_pool.tile([P, QB, DS, NB], bf)
            xn = xn_pool.tile([P, SKT, D], bf)
            mT = mT_pool.tile([P, 2, DS, NB], bf)
            w2T = w2T_pool.tile([P, DS, D], bf)

            S1 = stat_pool.tile([P, SQ], dt.float32)
            # ones (col 0) and 1/Z per query tile (cols 8..) share one tile
            zs = stat_pool.tile([P, 8 + SQT], dt.float32, name="zs")

            # PE warmup; memset on GpSimd which wakes earliest. 18 junk MMs
            # bridge engine-ready (~6.2us) to first-data (~8.4us) and start
            # the HAM busy window so real MMs run warm sooner.
            wrm = stat_pool.tile([P, P], bf, name="wrm")
            nc.gpsimd.memset(wrm[:], 0.0)
            nc.vector.memset(zs[:, 0:1], 1.0)
            wps = z_pool.tile([P, P], dt.float32, tag="zp", name="wps")
            for i in range(24):
                nc.tensor.matmul(wps[:], wrm[:], wrm[:], start=True, stop=True)

            # DMA in consumption order, all chunks with >=2KB contiguous
            # per-partition runs (1KB-run transfers halve DMA bandwidth).
            # Each dma_start is a ~600ns DMA_DIRECT2D on its issuing queue,
            # so spread triggers across four queues to issue in parallel:
            # sync: mT, gpsimd: xT, vector: xn, scalar: w2T.
            for dsp in range(DS // 2):
                nc.sync.dma_start(mT[:, 0, 2 * dsp:2 * dsp + 2, :],
                                  mT_d[:, 0, 2 * dsp:2 * dsp + 2, :])
                nc.gpsimd.dma_start(xT[:, 0, 2 * dsp:2 * dsp + 2, :],
                                    xT_d[:, 0, 2 * dsp:2 * dsp + 2, :])
            for dsq in range(DS // 4):
                nc.sync.dma_start(mT[:, 1, 4 * dsq:4 * dsq + 4, :],
                                  mT_d[:, 1, 4 * dsq:4 * dsq + 4, :])
            for dsq in range(DS // 4):
                nc.gpsimd.dma_start(xT[:, 1, 4 * dsq:4 * dsq + 4, :],
                                    xT_d[:, 1, 4 * dsq:4 * dsq + 4, :])
            for qb in range(2, QB):
                for dsq in range(DS // 4):
                    nc.gpsimd.dma_start(xT[:, qb, 4 * dsq:4 * dsq + 4, :],
                                        xT_d[:, qb, 4 * dsq:4 * dsq + 4, :])
            for tq in range(SKT // 4):
                nc.scalar.dma_start(xn[:, 4 * tq:4 * tq + 4, :],
                                    xn_d[:, 4 * tq:4 * tq + 4, :])
            for dsq in range(DS // 4):
                nc.sync.dma_start(w2T[:, 4 * dsq:4 * dsq + 4, :],
                                  w2T_d[:, 4 * dsq:4 * dsq + 4, :])

            uT = uT_pool.tile([P, DS, SQ], bf)
            NBq = min(NB, SQ)
            DTG = min(4, DS)

            def ut_half(qb):
                # uT[d', sq_qb] = sum_d mT[d, d'] xT[d, sq_qb]
                for g in range(DS // DTG):
                    grp = range(g * DTG, (g + 1) * DTG)
                    pss = [ps_pool.tile([P, NBq], dt.float32, tag="ps",
                                        name=f"ps_u{qb}_{g}_{j}")
                           for j in range(DTG)]
                    for ds in range(DS):
                        for j, dt_ in enumerate(grp):
                            nc.tensor.matmul(
                                pss[j][:], mT[:, dt_ // 4, ds,
                                              (dt_ % 4) * P:(dt_ % 4 + 1) * P],
                                xT[:, qb, ds, :],
                                start=(ds == 0), stop=(ds == DS - 1),
                            )
                    for j, dt_ in enumerate(grp):
                        nc.any.tensor_copy(
                            uT[:, dt_, qb * NBq:(qb + 1) * NBq], pss[j][:])

            pT = pT_pool.tile([P, SKT, SQ], bf)

            def scores_half(h):
                hq = h * HW_
                for skt in range(SKT):
                    ps1 = ps_pool.tile([P, HW_], dt.float32, tag="ps",
                                       name=f"ps_s{h}_{skt}")
                    for ds in range(DS):
                        nc.tensor.matmul(
                            ps1[:], xT[:, skt // 4, ds,
                                       (skt % 4) * P:(skt % 4 + 1) * P],
                            uT[:, ds, hq:hq + HW_],
                            start=(ds == 0), stop=(ds == DS - 1),
                        )
                    nc.scalar.activation(
                        pT[:, skt, hq:hq + HW_], ps1[:],
                        mybir.ActivationFunctionType.Exp, scale=INV_SQRT_D,
                    )
                    if skt == 0:
                        nc.vector.tensor_copy(S1[:, hq:hq + HW_],
                                              pT[:, 0, hq:hq + HW_])
                    else:
                        nc.vector.tensor_add(S1[:, hq:hq + HW_],
                                             S1[:, hq:hq + HW_],
                                             pT[:, skt, hq:hq + HW_])

            def z_half(h):
                for t in range(HT):
                    sqt = h * HT + t
                    zp = z_pool.tile([P, 1], dt.float32, tag="zp",
                                     name=f"zp{sqt}")
                    nc.tensor.matmul(zp[:], S1[:, sqt * P:(sqt + 1) * P],
                                     zs[:, 0:1], start=True, stop=True)
                    nc.vector.reciprocal(zs[:, 8 + sqt:9 + sqt], zp[:])

            px = px_pool.tile([P, DS, SQ], bf)

            def px_half(h):
                # pxT[d, sq_h] = sum_sk xn[sk, d] pT[sk, sq_h]
                hq = h * HW_
                for dt_ in range(DS):
                    ps2 = ps_pool.tile([P, HW_], dt.float32, tag="ps",
                                       name=f"ps_c{h}_{dt_}")
                    for skt in range(SKT):
                        nc.tensor.matmul(
                            ps2[:], xn[:, skt, dt_ * P:(dt_ + 1) * P],
                            pT[:, skt, hq:hq + HW_],
                            start=(skt == 0), stop=(skt == SKT - 1),
                        )
                    nc.any.tensor_copy(px[:, dt_, hq:hq + HW_], ps2[:])

            def emit_out_scale(sqt, col0, w, ps, eng):
                ot = out_pool.tile([P, w], dt.float32, tag="ot",
                                   name=f"ot{sqt}_{col0}")
                if eng == 0:
                    nc.vector.tensor_mul(
                        ot[:], ps[:],
                        zs[:, 8 + sqt:9 + sqt].to_broadcast([P, w]))
                else:
                    # same multiply on Scalar so adjacent blocks of a
                    # tile don't serialize on Vector
                    nc.scalar.activation(
                        ot[:], ps[:],
                        mybir.ActivationFunctionType.Copy,
                        scale=zs[:, 8 + sqt:9 + sqt])
                if eng == 0:
                    nc.sync.dma_start(
                        y_d[sqt * P:(sqt + 1) * P, col0:col0 + w], ot[:])
                else:
                    # alternate trigger queue so back-to-back out blocks
                    # don't serialize on one queue's ~600ns DMA_DIRECT2D
                    nc.gpsimd.dma_start(
                        y_d[sqt * P:(sqt + 1) * P, col0:col0 + w], ot[:])

            def out_half(h):
                for t in range(HT):
                    sqt = h * HT + t
                    last = h == NH - 1 and t == HT - 1
                    if last:
                        # final tile: tapered serial chains so each block's
                        # scale+DMA overlaps the next block's matmuls and
                        # the very last block's epilogue is minimal
                        col0 = 0
                        for fb, w in enumerate([512, 256, 128, 128]):
                            psf = ps_pool.tile([P, w], dt.float32,
                                               tag="ps", name=f"ps_of{fb}")
                            for ds in range(DS):
                                nc.tensor.matmul(
                                    psf[:],
                                    px[:, ds, sqt * P:(sqt + 1) * P],
                                    w2T[:, ds, col0:col0 + w],
                                    start=(ds == 0), stop=(ds == DS - 1),
                                )
                            emit_out_scale(sqt, col0, w, psf, fb % 2)
                            col0 += w
                    else:
                        pss = [ps_pool.tile([P, NBg], dt.float32, tag="ps",
                                            name=f"ps_o{sqt}_{i}")
                               for i in range(D // NBg)]
                        for ds in range(DS):
                            lhs = px[:, ds, sqt * P:(sqt + 1) * P]
                            for gb in range(D // NBg):
                                nc.tensor.matmul(
                                    pss[gb][:], lhs,
                                    w2T[:, ds, gb * NBg:(gb + 1) * NBg],
                                    start=(ds == 0), stop=(ds == DS - 1),
                                )
                        for gb in range(D // NBg):
                            emit_out_scale(sqt, gb * NBg, NBg, pss[gb],
                                           gb % 2)

            ut_half(0)
            scores_half(0)
            ut_half(1)
            z_half(0)
            px_half(0)
            scores_half(1)
            out_half(0)
            z_half(1)
            px_half(1)
            out_half(1)

    nc.compile()
    _BUILD_CACHE[key] = nc
    return nc


def _run(x, wq, wk, wv, wo, trace=False):
    from concourse.bass_utils import run_bass_kernel_spmd

    B, S, D = x.shape
    SQ = B * S // N_CORES
    halves = S // SQ
    DS = D // P
    SKT = S // P
    nc = _build(S, D, SQ)

    bf = ml_dtypes.bfloat16
    x = np.asarray(x, dtype=np.float32)
    M = (np.asarray(wq, np.float32).T @ np.asarray(wk, np.float32))
    W2 = (np.asarray(wo, np.float32) @ np.asarray(wv, np.float32))
    # M [k, n] -> [128, 2, k/128, 512]: k on partitions, col-half-major
    mT = np.ascontiguousarray(
        M.reshape(DS, P, 2, D // 2).transpose(1, 2, 0, 3)).astype(bf)
    w2T = np.ascontiguousarray(
        W2.T.reshape(DS, P, D).transpose(1, 0, 2)).astype(bf)

    in_maps = []
    for c in range(N_CORES):
        b, h = divmod(c, halves)
        xb = x[b]
        if h != 0:
            xb = np.concatenate([xb[h * SQ:(h + 1) * SQ], xb[:h * SQ],
                                 xb[(h + 1) * SQ:]], axis=0)
        xb = np.ascontiguousarray(xb, dtype=np.float32)
        # x.T, query-block-major: [128, QB, DS, 512] with d on partitions
        QB = S // 512
        xT = np.ascontiguousarray(
            xb.T.reshape(DS, P, QB, 512).transpose(1, 2, 0, 3)).astype(bf)
        # natural x, keys on partitions: [128, SKT, D]
        xn = np.ascontiguousarray(
            xb.reshape(SKT, P, D).transpose(1, 0, 2)).astype(bf)
        in_maps.append({"xT": xT, "xn": xn, "mT": mT, "w2T": w2T})

    res = run_bass_kernel_spmd(nc, in_maps, core_ids=list(range(N_CORES)),
                               trace=trace)
    out = np.empty((B, S, D), dtype=np.float32)
    for c in range(N_CORES):
        b, h = divmod(c, halves)
        out[b, h * SQ:(h + 1) * SQ, :] = res.results[c]["y"]
    return out, res


def kernel(x, wq, wk, wv, wo):
    out, _ = _run(x, wq, wk, wv, wo)
    return out



# revision 20
# speedup vs baseline: 1.0770x; 1.0770x over previous
"""Trainium2 Bass kernel for full-dim attention — bf16 resident v3.

Folded algorithm (scores = x M x.T / 32 with M = wq.T wk;
out = (p x) W2.T with W2 = wo wv). All matmul operands bf16 (same PE
rate as fp32r, half the SBUF/DMA), which lets every input stay fully
resident: x.T [128,8,2048], x natural [128,16,1024], mT, w2T, uT, pT
(both halves), px. Total ~21 MB SBUF, 12 MB DMA in, all front-loaded —
no mid-kernel streaming bursts (the v2 restructure showed the fp32r
xTk/xn re-streams run at ~270 GB/s demand and stall the PE).

Stage order (PE): uT(q0) S0 uT(q1) Z0 px0 S1 out0 Z1 px1 out1 — every
cross-engine handoff (psum evac, exp, S1 sums, Z) hides behind a PE
stage that doesn't depend on it.
"""

import sys

if "/opt/trn_rl_repo" not in sys.path:
    sys.path.insert(0, "/opt/trn_rl_repo")

import numpy as np
import ml_dtypes

N_CORES = 8
P = 128

_BUILD_CACHE = {}


def _build(S, D, SQ):
    import concourse.mybir as mybir
    import concourse.tile as tile
    from concourse import bacc

    key = (S, D, SQ)
    if key in _BUILD_CACHE:
        return _BUILD_CACHE[key]

    dt = mybir.dt
    DS = D // P           # d subtiles (8)
    SKT = S // P          # key tiles (16)
    SQT = SQ // P         # query tiles (8)
    NB = 512
    NBg = min(NB, D)      # 512
    NH = max(1, SQ // NB)          # query halves (2)
    HW_ = SQ // NH                 # queries per half (512)
    HT = HW_ // P                  # query tiles per half (4)
    QB = S // NB                   # 512-query blocks over all keys (4)
    INV_SQRT_D = 1.0 / float(np.sqrt(np.float32(D)))

    nc = bacc.Bacc(None, target_bir_lowering=False, debug=False)

    bf = dt.bfloat16
    # xT is query-block-major [P, QB, DS, 512]: a chunk [:, qb, ds0:ds1, :]
    # is contiguous >=2KB per partition. 1KB-run DMA transfers move at only
    # ~12.3 GB/s/engine (81ns/packet overhead-bound); 2KB+ runs hit ~24.5,
    # i.e. ~390 GB/s aggregate over the 16 engines.
    xT_d = nc.dram_tensor("xT", [P, QB, DS, NB], bf, kind="ExternalInput")
    xn_d = nc.dram_tensor("xn", [P, SKT, D], bf, kind="ExternalInput")
    # mT is col-half-major [P, 2, DS, 512]: uT group g consumes col-half g
    # across all ds, so chunk [:, h, ds0:ds1, :] is both contiguous (>=2KB
    # runs) and exactly what the next group needs.
    mT_d = nc.dram_tensor("mT", [P, 2, DS, NB], bf, kind="ExternalInput")
    w2T_d = nc.dram_tensor("w2T", [P, DS, D], bf, kind="ExternalInput")
    y_d = nc.dram_tensor("y", [SQ, D], dt.float32, kind="ExternalOutput")

    with tile.TileContext(nc) as tc:
        with (
            tc.tile_pool(name="xT", bufs=1) as xT_pool,
            tc.tile_pool(name="xn", bufs=1) as xn_pool,
            tc.tile_pool(name="mT", bufs=1) as mT_pool,
            tc.tile_pool(name="w2T", bufs=1) as w2T_pool,
            tc.tile_pool(name="uT", bufs=1) as uT_pool,
            tc.tile_pool(name="pT", bufs=1) as pT_pool,
            tc.tile_pool(name="px", bufs=1) as px_pool,
            tc.tile_pool(name="stat", bufs=1) as stat_pool,
            tc.tile_pool(name="outsb", bufs=4) as out_pool,
            tc.tile_pool(name="ps", bufs=6, space="PSUM") as ps_pool,
            tc.tile_pool(name="zps", bufs=2, space="PSUM") as z_pool,
        ):
            xT = xT_pool.tile([P, QB, DS, NB], bf)
            xn = xn_pool.tile([P, SKT, D], bf)
            mT = mT_pool.tile([P, 2, DS, NB], bf)
            w2T = w2T_pool.tile([P, DS, D], bf)

            S1 = stat_pool.tile([P, SQ], dt.float32)
            # ones (col 0) and 1/Z per query tile (cols 8..) share one tile
            zs = stat_pool.tile([P, 8 + SQT], dt.float32, name="zs")

            # PE warmup; memset on GpSimd which wakes earliest. 18 junk MMs
            # bridge engine-ready (~6.2us) to first-data (~8.4us) and start
            # the HAM busy window so real MMs run warm sooner.
            wrm = stat_pool.tile([P, P], bf, name="wrm")
            nc.gpsimd.memset(wrm[:], 0.0)
            nc.vector.memset(zs[:, 0:1], 1.0)
            wps = z_pool.tile([P, P], dt.float32, tag="zp", name="wps")
            for i in range(38):
                nc.tensor.matmul(wps[:], wrm[:], wrm[:], start=True, stop=True)

            # DMA in consumption order, all on the sync queue so issue
            # order == consumption order (parallel-queue triggers let late
            # tensors steal early bandwidth; ~600ns DMA_DIRECT2D per chunk
            # is just above the transfer rate, so serial is fine). All
            # chunks have >=2KB contiguous per-partition runs (1KB-run
            # transfers halve DMA bandwidth).
            for dsp in range(DS // 2):
                nc.sync.dma_start(mT[:, 0, 2 * dsp:2 * dsp + 2, :],
                                  mT_d[:, 0, 2 * dsp:2 * dsp + 2, :])
                nc.sync.dma_start(xT[:, 0, 2 * dsp:2 * dsp + 2, :],
                                  xT_d[:, 0, 2 * dsp:2 * dsp + 2, :])
            for dsq in range(DS // 4):
                nc.sync.dma_start(mT[:, 1, 4 * dsq:4 * dsq + 4, :],
                                  mT_d[:, 1, 4 * dsq:4 * dsq + 4, :])
            for dsq in range(DS // 4):
                nc.sync.dma_start(xT[:, 1, 4 * dsq:4 * dsq + 4, :],
                                  xT_d[:, 1, 4 * dsq:4 * dsq + 4, :])
            for qb in range(2, QB):
                for dsq in range(DS // 4):
                    nc.sync.dma_start(xT[:, qb, 4 * dsq:4 * dsq + 4, :],
                                      xT_d[:, qb, 4 * dsq:4 * dsq + 4, :])
            for tq in range(SKT // 4):
                nc.sync.dma_start(xn[:, 4 * tq:4 * tq + 4, :],
                                  xn_d[:, 4 * tq:4 * tq + 4, :])
            for dsq in range(DS // 4):
                nc.sync.dma_start(w2T[:, 4 * dsq:4 * dsq + 4, :],
                                  w2T_d[:, 4 * dsq:4 * dsq + 4, :])

            uT = uT_pool.tile([P, DS, SQ], bf)
            NBq = min(NB, SQ)
            DTG = min(4, DS)

            def ut_half(qb):
                # uT[d', sq_qb] = sum_d mT[d, d'] xT[d, sq_qb]
                for g in range(DS // DTG):
                    grp = range(g * DTG, (g + 1) * DTG)
                    pss = [ps_pool.tile([P, NBq], dt.float32, tag="ps",
                                        name=f"ps_u{qb}_{g}_{j}")
                           for j in range(DTG)]
                    for ds in range(DS):
                        for j, dt_ in enumerate(grp):
                            nc.tensor.matmul(
                                pss[j][:], mT[:, dt_ // 4, ds,
                                              (dt_ % 4) * P:(dt_ % 4 + 1) * P],
                                xT[:, qb, ds, :],
                                start=(ds == 0), stop=(ds == DS - 1),
                            )
                    for j, dt_ in enumerate(grp):
                        nc.any.tensor_copy(
                            uT[:, dt_, qb * NBq:(qb + 1) * NBq], pss[j][:])

            pT = pT_pool.tile([P, SKT, SQ], bf)

            def scores_half(h):
                hq = h * HW_
                for skt in range(SKT):
                    ps1 = ps_pool.tile([P, HW_], dt.float32, tag="ps",
                                       name=f"ps_s{h}_{skt}")
                    for ds in range(DS):
                        nc.tensor.matmul(
                            ps1[:], xT[:, skt // 4, ds,
                                       (skt % 4) * P:(skt % 4 + 1) * P],
                            uT[:, ds, hq:hq + HW_],
                            start=(ds == 0), stop=(ds == DS - 1),
                        )
                    nc.scalar.activation(
                        pT[:, skt, hq:hq + HW_], ps1[:],
                        mybir.ActivationFunctionType.Exp, scale=INV_SQRT_D,
                    )
                    if skt == 0:
                        nc.vector.tensor_copy(S1[:, hq:hq + HW_],
                                              pT[:, 0, hq:hq + HW_])
                    else:
                        nc.vector.tensor_add(S1[:, hq:hq + HW_],
                                             S1[:, hq:hq + HW_],
                                             pT[:, skt, hq:hq + HW_])

            def z_half(h):
                for t in range(HT):
                    sqt = h * HT + t
                    zp = z_pool.tile([P, 1], dt.float32, tag="zp",
                                     name=f"zp{sqt}")
                    nc.tensor.matmul(zp[:], S1[:, sqt * P:(sqt + 1) * P],
                                     zs[:, 0:1], start=True, stop=True)
                    nc.vector.reciprocal(zs[:, 8 + sqt:9 + sqt], zp[:])

            px = px_pool.tile([P, DS, SQ], bf)

            def px_half(h):
                # pxT[d, sq_h] = sum_sk xn[sk, d] pT[sk, sq_h]
                hq = h * HW_
                for dt_ in range(DS):
                    ps2 = ps_pool.tile([P, HW_], dt.float32, tag="ps",
                                       name=f"ps_c{h}_{dt_}")
                    for skt in range(SKT):
                        nc.tensor.matmul(
                            ps2[:], xn[:, skt, dt_ * P:(dt_ + 1) * P],
                            pT[:, skt, hq:hq + HW_],
                            start=(skt == 0), stop=(skt == SKT - 1),
                        )
                    nc.any.tensor_copy(px[:, dt_, hq:hq + HW_], ps2[:])

            def emit_out_scale(sqt, col0, w, ps, eng):
                ot = out_pool.tile([P, w], dt.float32, tag="ot",
                                   name=f"ot{sqt}_{col0}")
                if eng == 0:
                    nc.vector.tensor_mul(
                        ot[:], ps[:],
                        zs[:, 8 + sqt:9 + sqt].to_broadcast([P, w]))
                else:
                    # same multiply on Scalar so adjacent blocks of a
                    # tile don't serialize on Vector
                    nc.scalar.activation(
                        ot[:], ps[:],
                        mybir.ActivationFunctionType.Copy,
                        scale=zs[:, 8 + sqt:9 + sqt])
                if eng == 0:
                    nc.sync.dma_start(
                        y_d[sqt * P:(sqt + 1) * P, col0:col0 + w], ot[:])
                else:
                    # alternate trigger queue so back-to-back out blocks
                    # don't serialize on one queue's ~600ns DMA_DIRECT2D
                    nc.gpsimd.dma_start(
                        y_d[sqt * P:(sqt + 1) * P, col0:col0 + w], ot[:])

            def out_half(h):
                for t in range(HT):
                    sqt = h * HT + t
                    last = h == NH - 1 and t == HT - 1
                    if last:
                        # final tile: tapered serial chains so each block's
                        # scale+DMA overlaps the next block's matmuls and
                        # the very last block's epilogue is minimal
                        col0 = 0
                        for fb, w in enumerate([512, 256, 128, 128]):
                            psf = ps_pool.tile([P, w], dt.float32,
                                               tag="ps", name=f"ps_of{fb}")
                            for ds in range(DS):
                                nc.tensor.matmul(
                                    psf[:],
                                    px[:, ds, sqt * P:(sqt + 1) * P],
                                    w2T[:, ds, col0:col0 + w],
                                    start=(ds == 0), stop=(ds == DS - 1),
                                )
                            emit_out_scale(sqt, col0, w, psf, fb % 2)
                            col0 += w
                    else:
                        pss = [ps_pool.tile([P, NBg], dt.float32, tag="ps",
                                            name=f"ps_o{sqt}_{i}")
                               for i in range(D // NBg)]
                        for ds in range(DS):
                            lhs = px[:, ds, sqt * P:(sqt + 1) * P]
                            for gb in range(D // NBg):
                                nc.tensor.matmul(
                                    pss[gb][:], lhs,
                                    w2T[:, ds, gb * NBg:(gb + 1) * NBg],
                                    start=(ds == 0), stop=(ds == DS - 1),
                                )
                        for gb in range(D // NBg):
                            emit_out_scale(sqt, gb * NBg, NBg, pss[gb],
                                           gb % 2)

            ut_half(0)
            scores_half(0)
            ut_half(1)
            z_half(0)
            px_half(0)
            scores_half(1)
            out_half(0)
            z_half(1)
            px_half(1)
            out_half(1)

    nc.compile()
    _BUILD_CACHE[key] = nc
    return nc


def _run(x, wq, wk, wv, wo, trace=False):
    from concourse.bass_utils import run_bass_kernel_spmd

    B, S, D = x.shape
    SQ = B * S // N_CORES
    halves = S // SQ
    DS = D // P
    SKT = S // P
    nc = _build(S, D, SQ)

    bf = ml_dtypes.bfloat16
    x = np.asarray(x, dtype=np.float32)
    M = (np.asarray(wq, np.float32).T @ np.asarray(wk, np.float32))
    W2 = (np.asarray(wo, np.float32) @ np.asarray(wv, np.float32))
    # M [k, n] -> [128, 2, k/128, 512]: k on partitions, col-half-major
    mT = np.ascontiguousarray(
        M.reshape(DS, P, 2, D // 2).transpose(1, 2, 0, 3)).astype(bf)
    w2T = np.ascontiguousarray(
        W2.T.reshape(DS, P, D).transpose(1, 0, 2)).astype(bf)

    in_maps = []
    for c in range(N_CORES):
        b, h = divmod(c, halves)
        xb = x[b]
        if h != 0:
            xb = np.concatenate([xb[h * SQ:(h + 1) * SQ], xb[:h * SQ],
                                 xb[(h + 1) * SQ:]], axis=0)
        xb = np.ascontiguousarray(xb, dtype=np.float32)
        # x.T, query-block-major: [128, QB, DS, 512] with d on partitions
        QB = S // 512
        xT = np.ascontiguousarray(
            xb.T.reshape(DS, P, QB, 512).transpose(1, 2, 0, 3)).astype(bf)
        # natural x, keys on partitions: [128, SKT, D]
        xn = np.ascontiguousarray(
            xb.reshape(SKT, P, D).transpose(1, 0, 2)).astype(bf)
        in_maps.append({"xT": xT, "xn": xn, "mT": mT, "w2T": w2T})

    res = run_bass_kernel_spmd(nc, in_maps, core_ids=list(range(N_CORES)),
                               trace=trace)
    out = np.empty((B, S, D), dtype=np.float32)
    for c in range(N_CORES):
        b, h = divmod(c, halves)
        out[b, h * SQ:(h + 1) * SQ, :] = res.results[c]["y"]
    return out, res


def kernel(x, wq, wk, wv, wo):
    out, _ = _run(x, wq, wk, wv, wo)
    return out



# revision 22
# speedup vs baseline: 1.0795x; 1.0023x over previous
"""Trainium2 Bass kernel for full-dim attention — bf16 resident v3.

Folded algorithm (scores = x M x.T / 32 with M = wq.T wk;
out = (p x) W2.T with W2 = wo wv). All matmul operands bf16 (same PE
rate as fp32r, half the SBUF/DMA), which lets every input stay fully
resident: x.T [128,8,2048], x natural [128,16,1024], mT, w2T, uT, pT
(both halves), px. Total ~21 MB SBUF, 12 MB DMA in, all front-loaded —
no mid-kernel streaming bursts (the v2 restructure showed the fp32r
xTk/xn re-streams run at ~270 GB/s demand and stall the PE).

Stage order (PE): uT(q0) S0 uT(q1) Z0 px0 S1 out0 Z1 px1 out1 — every
cross-engine handoff (psum evac, exp, S1 sums, Z) hides behind a PE
stage that doesn't depend on it.
"""

import sys

if "/opt/trn_rl_repo" not in sys.path:
    sys.path.insert(0, "/opt/trn_rl_repo")

import numpy as np
import ml_dtypes

N_CORES = 8
P = 128

_BUILD_CACHE = {}


def _build(S, D, SQ):
    import concourse.mybir as mybir
    import concourse.tile as tile
    from concourse import bacc

    key = (S, D, SQ)
    if key in _BUILD_CACHE:
        return _BUILD_CACHE[key]

    dt = mybir.dt
    DS = D // P           # d subtiles (8)
    SKT = S // P          # key tiles (16)
    SQT = SQ // P         # query tiles (8)
    NB = 512
    NBg = min(NB, D)      # 512
    NH = max(1, SQ // NB)          # query halves (2)
    HW_ = SQ // NH                 # queries per half (512)
    HT = HW_ // P                  # query tiles per half (4)
    QB = S // NB                   # 512-query blocks over all keys (4)
    INV_SQRT_D = 1.0 / float(np.sqrt(np.float32(D)))

    nc = bacc.Bacc(None, target_bir_lowering=False, debug=False)

    bf = dt.bfloat16
    # xT is query-block-major [P, QB, DS, 512]: a chunk [:, qb, ds0:ds1, :]
    # is contiguous >=2KB per partition. 1KB-run DMA transfers move at only
    # ~12.3 GB/s/engine (81ns/packet overhead-bound); 2KB+ runs hit ~24.5,
    # i.e. ~390 GB/s aggregate over the 16 engines.
    xT_d = nc.dram_tensor("xT", [P, QB, DS, NB], bf, kind="ExternalInput")
    xn_d = nc.dram_tensor("xn", [P, SKT, D], bf, kind="ExternalInput")
    # mT is col-half-major [P, 2, DS, 512]: uT group g consumes col-half g
    # across all ds, so chunk [:, h, ds0:ds1, :] is both contiguous (>=2KB
    # runs) and exactly what the next group needs.
    mT_d = nc.dram_tensor("mT", [P, 2, DS, NB], bf, kind="ExternalInput")
    w2T_d = nc.dram_tensor("w2T", [P, DS, D], bf, kind="ExternalInput")
    y_d = nc.dram_tensor("y", [SQ, D], dt.float32, kind="ExternalOutput")

    with tile.TileContext(nc) as tc:
        with (
            tc.tile_pool(name="xT", bufs=1) as xT_pool,
            tc.tile_pool(name="xn", bufs=1) as xn_pool,
            tc.tile_pool(name="mT", bufs=1) as mT_pool,
            tc.tile_pool(name="w2T", bufs=1) as w2T_pool,
            tc.tile_pool(name="uT", bufs=1) as uT_pool,
            tc.tile_pool(name="pT", bufs=1) as pT_pool,
            tc.tile_pool(name="px", bufs=1) as px_pool,
            tc.tile_pool(name="stat", bufs=1) as stat_pool,
            tc.tile_pool(name="outsb", bufs=4) as out_pool,
            tc.tile_pool(name="ps", bufs=6, space="PSUM") as ps_pool,
            tc.tile_pool(name="zps", bufs=2, space="PSUM") as z_pool,
        ):
            xT = xT_pool.tile([P, QB, DS, NB], bf)
            xn = xn_pool.tile([P, SKT, D], bf)
            mT = mT_pool.tile([P, 2, DS, NB], bf)
            w2T = w2T_pool.tile([P, DS, D], bf)

            S1 = stat_pool.tile([P, SQ], dt.float32)
            # ones (col 0) and 1/Z per query tile (cols 8..) share one tile
            zs = stat_pool.tile([P, 8 + SQT], dt.float32, name="zs")

            # PE warmup; memset on GpSimd which wakes earliest. 18 junk MMs
            # bridge engine-ready (~6.2us) to first-data (~8.4us) and start
            # the HAM busy window so real MMs run warm sooner.
            wrm = stat_pool.tile([P, P], bf, name="wrm")
            nc.gpsimd.memset(wrm[:], 0.0)
            nc.vector.memset(zs[:, 0:1], 1.0)
            wps = z_pool.tile([P, P], dt.float32, tag="zp", name="wps")
            for i in range(38):
                nc.tensor.matmul(wps[:], wrm[:], wrm[:], start=True, stop=True)

            # DMA in consumption order, all on the sync queue so issue
            # order == consumption order (parallel-queue triggers let late
            # tensors steal early bandwidth; ~600ns DMA_DIRECT2D per chunk
            # is just above the transfer rate, so serial is fine). All
            # chunks have >=2KB contiguous per-partition runs (1KB-run
            # transfers halve DMA bandwidth).
            for dsp in range(DS // 2):
                nc.sync.dma_start(mT[:, 0, 2 * dsp:2 * dsp + 2, :],
                                  mT_d[:, 0, 2 * dsp:2 * dsp + 2, :])
                nc.sync.dma_start(xT[:, 0, 2 * dsp:2 * dsp + 2, :],
                                  xT_d[:, 0, 2 * dsp:2 * dsp + 2, :])
            for dsq in range(DS // 4):
                nc.sync.dma_start(mT[:, 1, 4 * dsq:4 * dsq + 4, :],
                                  mT_d[:, 1, 4 * dsq:4 * dsq + 4, :])
            for dsq in range(DS // 4):
                nc.sync.dma_start(xT[:, 1, 4 * dsq:4 * dsq + 4, :],
                                  xT_d[:, 1, 4 * dsq:4 * dsq + 4, :])
            for qb in range(2, QB):
                for dsq in range(DS // 4):
                    nc.sync.dma_start(xT[:, qb, 4 * dsq:4 * dsq + 4, :],
                                      xT_d[:, qb, 4 * dsq:4 * dsq + 4, :])
            for tq in range(SKT // 4):
                nc.sync.dma_start(xn[:, 4 * tq:4 * tq + 4, :],
                                  xn_d[:, 4 * tq:4 * tq + 4, :])
            for dsq in range(DS // 4):
                nc.sync.dma_start(w2T[:, 4 * dsq:4 * dsq + 4, :],
                                  w2T_d[:, 4 * dsq:4 * dsq + 4, :])

            uT = uT_pool.tile([P, DS, SQ], bf)
            NBq = min(NB, SQ)
            DTG = min(4, DS)

            def ut_half(qb):
                # uT[d', sq_qb] = sum_d mT[d, d'] xT[d, sq_qb]
                for g in range(DS // DTG):
                    grp = range(g * DTG, (g + 1) * DTG)
                    pss = [ps_pool.tile([P, NBq], dt.float32, tag="ps",
                                        name=f"ps_u{qb}_{g}_{j}")
                           for j in range(DTG)]
                    for ds in range(DS):
                        for j, dt_ in enumerate(grp):
                            nc.tensor.matmul(
                                pss[j][:], mT[:, dt_ // 4, ds,
                                              (dt_ % 4) * P:(dt_ % 4 + 1) * P],
                                xT[:, qb, ds, :],
                                start=(ds == 0), stop=(ds == DS - 1),
                            )
                    for j, dt_ in enumerate(grp):
                        nc.any.tensor_copy(
                            uT[:, dt_, qb * NBq:(qb + 1) * NBq], pss[j][:])

            pT = pT_pool.tile([P, SKT, SQ], bf)

            def scores_half(h):
                hq = h * HW_
                for skt in range(SKT):
                    ps1 = ps_pool.tile([P, HW_], dt.float32, tag="ps",
                                       name=f"ps_s{h}_{skt}")
                    for ds in range(DS):
                        nc.tensor.matmul(
                            ps1[:], xT[:, skt // 4, ds,
                                       (skt % 4) * P:(skt % 4 + 1) * P],
                            uT[:, ds, hq:hq + HW_],
                            start=(ds == 0), stop=(ds == DS - 1),
                        )
                    nc.scalar.activation(
                        pT[:, skt, hq:hq + HW_], ps1[:],
                        mybir.ActivationFunctionType.Exp, scale=INV_SQRT_D,
                    )
                    if skt == 0:
                        nc.vector.tensor_copy(S1[:, hq:hq + HW_],
                                              pT[:, 0, hq:hq + HW_])
                    else:
                        nc.vector.tensor_add(S1[:, hq:hq + HW_],
                                             S1[:, hq:hq + HW_],
                                             pT[:, skt, hq:hq + HW_])

            def z_half(h):
                for t in range(HT):
                    sqt = h * HT + t
                    zp = z_pool.tile([P, 1], dt.float32, tag="zp",
                                     name=f"zp{sqt}")
                    nc.tensor.matmul(zp[:], S1[:, sqt * P:(sqt + 1) * P],
                                     zs[:, 0:1], start=True, stop=True)
                    nc.vector.reciprocal(zs[:, 8 + sqt:9 + sqt], zp[:])

            px = px_pool.tile([P, DS, SQ], bf)

            def px_half(h):
                # pxT[d, sq_h] = sum_sk xn[sk, d] pT[sk, sq_h]
                hq = h * HW_
                for dt_ in range(DS):
                    ps2 = ps_pool.tile([P, HW_], dt.float32, tag="ps",
                                       name=f"ps_c{h}_{dt_}")
                    for skt in range(SKT):
                        nc.tensor.matmul(
                            ps2[:], xn[:, skt, dt_ * P:(dt_ + 1) * P],
                            pT[:, skt, hq:hq + HW_],
                            start=(skt == 0), stop=(skt == SKT - 1),
                        )
                    nc.any.tensor_copy(px[:, dt_, hq:hq + HW_], ps2[:])

            def emit_out_scale(sqt, col0, w, ps, eng):
                ot = out_pool.tile([P, w], dt.float32, tag="ot",
                                   name=f"ot{sqt}_{col0}")
                if eng == 0:
                    nc.vector.tensor_mul(
                        ot[:], ps[:],
                        zs[:, 8 + sqt:9 + sqt].to_broadcast([P, w]))
                else:
                    # same multiply on Scalar so adjacent blocks of a
                    # tile don't serialize on Vector
                    nc.scalar.activation(
                        ot[:], ps[:],
                        mybir.ActivationFunctionType.Copy,
                        scale=zs[:, 8 + sqt:9 + sqt])
                nc.sync.dma_start(
                    y_d[sqt * P:(sqt + 1) * P, col0:col0 + w], ot[:])

            def out_half(h):
                for t in range(HT):
                    sqt = h * HT + t
                    last = h == NH - 1 and t == HT - 1
                    if last:
                        # final tile: tapered serial chains so each block's
                        # scale+DMA overlaps the next block's matmuls and
                        # the very last block's epilogue is minimal
                        col0 = 0
                        for fb, w in enumerate([512, 256, 256]):
                            psf = ps_pool.tile([P, w], dt.float32,
                                               tag="ps", name=f"ps_of{fb}")
                            for ds in range(DS):
                                nc.tensor.matmul(
                                    psf[:],
                                    px[:, ds, sqt * P:(sqt + 1) * P],
                                    w2T[:, ds, col0:col0 + w],
                                    start=(ds == 0), stop=(ds == DS - 1),
                                )
                            emit_out_scale(sqt, col0, w, psf, fb % 2)
                            col0 += w
                    else:
                        pss = [ps_pool.tile([P, NBg], dt.float32, tag="ps",
                                            name=f"ps_o{sqt}_{i}")
                               for i in range(D // NBg)]
                        for ds in range(DS):
                            lhs = px[:, ds, sqt * P:(sqt + 1) * P]
                            for gb in range(D // NBg):
                                nc.tensor.matmul(
                                    pss[gb][:], lhs,
                                    w2T[:, ds, gb * NBg:(gb + 1) * NBg],
                                    start=(ds == 0), stop=(ds == DS - 1),
                                )
                        for gb in range(D // NBg):
                            emit_out_scale(sqt, gb * NBg, NBg, pss[gb],
                                           gb % 2)

            ut_half(0)
            scores_half(0)
            ut_half(1)
            z_half(0)
            px_half(0)
            scores_half(1)
            out_half(0)
            z_half(1)
            px_half(1)
            out_half(1)

    nc.compile()
    _BUILD_CACHE[key] = nc
    return nc


def _run(x, wq, wk, wv, wo, trace=False):
    from concourse.bass_utils import run_bass_kernel_spmd

    B, S, D = x.shape
    SQ = B * S // N_CORES
    halves = S // SQ
    DS = D // P
    SKT = S // P
    nc = _build(S, D, SQ)

    bf = ml_dtypes.bfloat16
    x = np.asarray(x, dtype=np.float32)
    M = (np.asarray(wq, np.float32).T @ np.asarray(wk, np.float32))
    W2 = (np.asarray(wo, np.float32) @ np.asarray(wv, np.float32))
    # M [k, n] -> [128, 2, k/128, 512]: k on partitions, col-half-major
    mT = np.ascontiguousarray(
        M.reshape(DS, P, 2, D // 2).transpose(1, 2, 0, 3)).astype(bf)
    w2T = np.ascontiguousarray(
        W2.T.reshape(DS, P, D).transpose(1, 0, 2)).astype(bf)

    in_maps = []
    for c in range(N_CORES):
        b, h = divmod(c, halves)
        xb = x[b]
        if h != 0:
            xb = np.concatenate([xb[h * SQ:(h + 1) * SQ], xb[:h * SQ],
                                 xb[(h + 1) * SQ:]], axis=0)
        xb = np.ascontiguousarray(xb, dtype=np.float32)
        # x.T, query-block-major: [128, QB, DS, 512] with d on partitions
        QB = S // 512
        xT = np.ascontiguousarray(
            xb.T.reshape(DS, P, QB, 512).transpose(1, 2, 0, 3)).astype(bf)
        # natural x, keys on partitions: [128, SKT, D]
        xn = np.ascontiguousarray(
            xb.reshape(SKT, P, D).transpose(1, 0, 2)).astype(bf)
        in_maps.append({"xT": xT, "xn": xn, "mT": mT, "w2T": w2T})

    res = run_bass_kernel_spmd(nc, in_maps, core_ids=list(range(N_CORES)),
                               trace=trace)
    out = np.empty((B, S, D), dtype=np.float32)
    for c in range(N_CORES):
        b, h = divmod(c, halves)
        out[b, h * SQ:(h + 1) * SQ, :] = res.results[c]["y"]
    return out, res


def kernel(x, wq, wk, wv, wo):
    out, _ = _run(x, wq, wk, wv, wo)
    return out

